# revision 3
# baseline (speedup 1.0000x reference)
"""Trainium2 Bass kernel: elementwise ive(49.5, z) on 8 cores — V2c (ACT3/DVE2).

Math: on the fp32-representable region (z >~ 7; below that exp underflows to
0/subnormal exactly like the fp32 reference), a weighted minimax fit gives
    ln ive(v,z) ~= D0 + D1*z + B*ln(z+b1) + C*ln(z+b2)
with max |error| = 3.24e-3 (ln(z+b2), b2~55, tracks ln(v+sqrt(v^2+z^2));
measured fp32 end-to-end: l2 rel 1.9e-3).

Per core (shard [512, 8192], tiles [128, F]):
    W1 = Ln(z + b1)                  ACT
    W2 = Ln(z + b2)                  ACT
    W2 = (D1/C)*z + W2               DVE STT (in-place)
    W1 = (B/C)*W1 + W2               DVE STT (in-place)
    o  = Exp(C*W1 + D0)              ACT (scale+bias)
All ACT funcs live in the natural_log_exp_and_others set: one table load.
"""

import numpy as np

# ---- fitted constants ----
D0 = 117.367419
D1 = 0.0770235055
B = 52.4329457
C = -75.6988139
B1 = 0.2187
B2 = 55.3181

N_CORES = 8
FULL_ROWS, COLS = 4096, 8192
ROWS = FULL_ROWS // N_CORES  # 512
P = 128
F = 2048
BUFS = dict(z=8, W1=6, W2=4, o=6)
OUT_DMA_ENGINE = "gpsimd"
# Per-row-group chunk widths; None -> uniform F. Keys: row group index or "mid".
# Small edge chunks shorten pipeline fill (first compute waits on less DMA)
# and the tail dependency chain after the last input lands.
SCHEDULE = {
    0: [512, 512, 1024, 2048, 2048, 2048],
    3: [2048, 2048, 2048, 1024, 512, 512],
    "mid": [2048, 2048, 2048, 2048],
}


def _chunks(rg):
    if SCHEDULE is None:
        widths = [F] * (COLS // F)
    else:
        widths = SCHEDULE.get(rg, SCHEDULE.get(str(rg), SCHEDULE["mid"]))
    assert sum(widths) == COLS, widths
    off = 0
    out = []
    for wd in widths:
        out.append((off, wd))
        off += wd
    return out

_CACHED_NC = None


def _build_nc():
    import concourse.bacc as bacc
    import concourse.bass as bass
    import concourse.tile as tile
    from concourse import mybir

    f32 = mybir.dt.float32
    AF = mybir.ActivationFunctionType
    OP = mybir.AluOpType

    # Steer every activation to the combined Ln+Exp table set so there is a
    # single table load (sets lacking either Ln or Exp are hidden; list order
    # kept so act_func_set_id indices stay valid).
    if not getattr(bacc, "_ive_act_tables_patched", False):
        _orig_get_tables = bacc.get_activation_tables
        _need = {AF.Ln, AF.Exp}

        def _patched_get_tables(arch):
            tabs = _orig_get_tables(arch)
            return {
                name: (fns if _need <= fns else set())
                for name, fns in tabs.items()
            }

        bacc.get_activation_tables = _patched_get_tables
        bacc._ive_act_tables_patched = True

    nc = bacc.Bacc("TRN2", target_bir_lowering=False, debug=False)
    # activation bias floats require pre-registered [128,1] const SBUF tensors
    for _v in (B1, B2, D0):
        _t = nc.alloc_sbuf_tensor(f"const-f32-{_v}", [128, 1], f32)
        nc.gpsimd.memset(_t.ap(), _v)
        nc.const_aps.aps[(f32, _v)] = _t.ap()
    nc.all_engine_barrier()
    z_d = nc.dram_tensor("z", [ROWS, COLS], f32, kind="ExternalInput").ap()
    o_d = nc.dram_tensor("out", [ROWS, COLS], f32, kind="ExternalOutput").ap()

    out_dma = getattr(nc, OUT_DMA_ENGINE).dma_start

    with tile.TileContext(nc) as tc:
        with tc.tile_pool(name="work", bufs=2) as pool:
            for rg in range(ROWS // P):
                for off, wd in _chunks(rg):
                    rs = bass.ts(rg, P)
                    cs = bass.ds(off, wd)

                    z = pool.tile([P, wd], f32, tag="z", bufs=BUFS["z"])
                    nc.sync.dma_start(out=z[:], in_=z_d[rs, cs])

                    W1 = pool.tile([P, wd], f32, tag="W1", bufs=BUFS["W1"])
                    nc.scalar.activation(W1[:], z[:], AF.Ln, bias=B1)

                    W2 = pool.tile([P, wd], f32, tag="W2", bufs=BUFS["W2"])
                    nc.scalar.activation(W2[:], z[:], AF.Ln, bias=B2)

                    # in-place: z <- (D1/C)*z + W2 (frees W2 early);
                    # W1 <- (B/C)*W1 + z (frees z)
                    nc.vector.scalar_tensor_tensor(
                        out=z[:], in0=z[:], scalar=D1 / C, in1=W2[:],
                        op0=OP.mult, op1=OP.add)
                    nc.vector.scalar_tensor_tensor(
                        out=W1[:], in0=W1[:], scalar=B / C, in1=z[:],
                        op0=OP.mult, op1=OP.add)

                    o = pool.tile([P, wd], f32, tag="o", bufs=BUFS["o"])
                    nc.scalar.activation(o[:], W1[:], AF.Exp, bias=D0, scale=C)

                    out_dma(out=o_d[rs, cs], in_=o[:])

    nc.compile()
    return nc


def kernel(z: np.ndarray) -> np.ndarray:
    global _CACHED_NC
    if _CACHED_NC is None:
        _CACHED_NC = _build_nc()
    nc = _CACHED_NC

    from concourse.bass_utils import run_bass_kernel_spmd

    z = np.ascontiguousarray(z, dtype=np.float32)
    shards = np.split(z, N_CORES, axis=0)
    in_maps = [{"z": np.ascontiguousarray(s)} for s in shards]
    res = run_bass_kernel_spmd(nc, in_maps, core_ids=list(range(N_CORES)))
    out = np.concatenate([res.results[i]["out"] for i in range(N_CORES)], axis=0)
    return np.ascontiguousarray(out, dtype=np.float32)


# revision 4
# speedup vs baseline: 1.0112x; 1.0112x over previous
"""Trainium2 Bass kernel: elementwise ive(49.5, z) = exp(-z)*I_v(z) on 8 cores.

Math: on the fp32-representable region (z >~ 7; below that the true result
underflows to 0/subnormal exactly like the fp32 reference), a weighted
minimax fit over z in [0.5, 99.5] gives
    ln ive(v,z) ~= D0 + D1*z + B*ln(z+b1) + C*ln(z+b2)
with max |error| 3.24e-3 (ln(z+b2), b2~55, tracks ln(v+sqrt(v^2+z^2)) of the
uniform asymptotic expansion; the fit is constrained to stay < -90 where the
reference underflows). Measured end-to-end fp32: l2 rel 1.8e-3.

Per core (shard [512, 8192] of the [4096, 8192] input, row-sharded), per
[128, wd] tile:
    W1 = Ln(z + b1)                  ACT
    W2 = Ln(z + b2)                  ACT
    z  = (D1/C)*z + W2               DVE STT (in-place, frees W2 early)
    W1 = (B/C)*W1 + z                DVE STT (in-place)
    o  = Exp(C*W1 + D0)              ACT (free scale+bias)
3 ACT passes (~92us/core) + 2 DVE passes (~70us/core) against a ~93us/core
DMA floor (16 MiB in + 16 MiB out at ~360 GB/s). All ACT funcs live in the
natural_log_exp_and_others table set: one table load.

Schedule: small chunks at the global start (shorter pipeline fill) and end
(shorter last-tile dependency chain after the final input lands); 4096-wide
compute chunks mid-kernel (lower ACT per-instruction overhead) fed by
2048-wide (1 MiB) sub-tile DMAs. Output DMAs ride the SWDGE ring (gpsimd)
so input (HWDGE/SP) and output issue don't serialize.
"""

import numpy as np

# ---- fitted constants ----
D0 = 117.367419
D1 = 0.0770235055
B = 52.4329457
C = -75.6988139
B1 = 0.2187
B2 = 55.3181

N_CORES = 8
FULL_ROWS, COLS = 4096, 8192
ROWS = FULL_ROWS // N_CORES  # 512 rows per core
P = 128

# (compute_width, [dma_widths]) per row group
SCHEDULE = {
    0: [(512, [512]), (512, [512]), (1024, [1024]), (2048, [2048]),
        (4096, [2048, 2048])],
    1: [(4096, [2048, 2048]), (4096, [2048, 2048])],
    2: [(4096, [2048, 2048]), (2048, [2048]), (2048, [2048])],
    3: [(2048, [2048]), (2048, [2048]), (1024, [1024]), (1024, [1024]),
        (1024, [1024]), (512, [512]), (512, [512])],
}
BUFS = dict(z=4, W1=3, W2=2, o=3)

_CACHED_NC = None


def _build_nc():
    import concourse.bacc as bacc
    import concourse.bass as bass
    import concourse.tile as tile
    from concourse import mybir

    f32 = mybir.dt.float32
    AF = mybir.ActivationFunctionType
    OP = mybir.AluOpType

    # Steer every activation to the combined Ln+Exp table set so there is a
    # single table load (sets lacking either Ln or Exp are hidden; list order
    # kept so act_func_set_id indices stay valid).
    if not getattr(bacc, "_ive_act_tables_patched", False):
        _orig_get_tables = bacc.get_activation_tables
        _need = {AF.Ln, AF.Exp}

        def _patched_get_tables(arch):
            tabs = _orig_get_tables(arch)
            return {
                name: (fns if _need <= fns else set())
                for name, fns in tabs.items()
            }

        bacc.get_activation_tables = _patched_get_tables
        bacc._ive_act_tables_patched = True

    nc = bacc.Bacc("TRN2", target_bir_lowering=False, debug=False)
    # activation bias floats require pre-registered [128,1] const SBUF tensors
    for _v in (B1, B2, D0):
        _t = nc.alloc_sbuf_tensor(f"const-f32-{_v}", [128, 1], f32)
        nc.gpsimd.memset(_t.ap(), _v)
        nc.const_aps.aps[(f32, _v)] = _t.ap()
    nc.all_engine_barrier()
    z_d = nc.dram_tensor("z", [ROWS, COLS], f32, kind="ExternalInput").ap()
    o_d = nc.dram_tensor("out", [ROWS, COLS], f32, kind="ExternalOutput").ap()

    with tile.TileContext(nc) as tc:
        with tc.tile_pool(name="work", bufs=2) as pool:
            for rg in range(ROWS // P):
                off = 0
                for wd, dmas in SCHEDULE[rg]:
                    rs = bass.ts(rg, P)

                    z = pool.tile([P, wd], f32, tag="z", bufs=BUFS["z"])
                    doff = 0
                    for dw in dmas:
                        nc.sync.dma_start(out=z[:, doff:doff + dw],
                                          in_=z_d[rs, bass.ds(off + doff, dw)])
                        doff += dw

                    W1 = pool.tile([P, wd], f32, tag="W1", bufs=BUFS["W1"])
                    nc.scalar.activation(W1[:], z[:], AF.Ln, bias=B1)

                    W2 = pool.tile([P, wd], f32, tag="W2", bufs=BUFS["W2"])
                    nc.scalar.activation(W2[:], z[:], AF.Ln, bias=B2)

                    # in-place: z <- (D1/C)*z + W2 (frees W2 early);
                    # W1 <- (B/C)*W1 + z (frees z)
                    nc.vector.scalar_tensor_tensor(
                        out=z[:], in0=z[:], scalar=D1 / C, in1=W2[:],
                        op0=OP.mult, op1=OP.add)
                    nc.vector.scalar_tensor_tensor(
                        out=W1[:], in0=W1[:], scalar=B / C, in1=z[:],
                        op0=OP.mult, op1=OP.add)

                    o = pool.tile([P, wd], f32, tag="o", bufs=BUFS["o"])
                    nc.scalar.activation(o[:], W1[:], AF.Exp, bias=D0, scale=C)

                    doff = 0
                    for dw in dmas:
                        nc.gpsimd.dma_start(out=o_d[rs, bass.ds(off + doff, dw)],
                                            in_=o[:, doff:doff + dw])
                        doff += dw
                    off += wd

    nc.compile()
    return nc


def kernel(z: np.ndarray) -> np.ndarray:
    global _CACHED_NC
    if _CACHED_NC is None:
        _CACHED_NC = _build_nc()
    nc = _CACHED_NC

    from concourse.bass_utils import run_bass_kernel_spmd

    z = np.ascontiguousarray(z, dtype=np.float32)
    shards = np.split(z, N_CORES, axis=0)
    in_maps = [{"z": np.ascontiguousarray(s)} for s in shards]
    res = run_bass_kernel_spmd(nc, in_maps, core_ids=list(range(N_CORES)))
    out = np.concatenate([res.results[i]["out"] for i in range(N_CORES)], axis=0)
    return np.ascontiguousarray(out, dtype=np.float32)
